# revision 34
# baseline (speedup 1.0000x reference)
"""Trainium2 Bass kernel for the moe_routing problem (nn_DAWN_69904887709893).

Token-parallel across 8 NeuronCores (256 tokens/core), neuron pools replicated.
The six neuron pools (f_qk/f_v/f_know, r_qk/r_v/r_know) stream from HBM in
fp8 (e4m3, x512 host prescale) and all feature/restore matmuls run in fp8
DoubleRow perf mode (2 contraction rows per PE cell), pairing adjacent
128-row k-tiles.  All scale factors are powers of two so rescaling is exact:
nx is quantized x16, the restore route weights carry 1/128, and every restore
PSUM holds 32768*y, folded out in the PSUM->SBUF copy.  Restores are
token-major (stationary = fp8 token-side operand, moving = fp8 r chunk,
full-bank [128,512] accumulators with clean start/stop), so the knowledge
restore needs no final transposes; Q/K are transposed + quantized to fp8
after the fused Q+K restore, K^T is AllGathered in fp8 (half the wire bytes)
while the V restore computes, and V gathers token-major fp8.  Matmuls feeding
router logits (W_all, logit projections, W_fk/W_rk) stay in plain fp32 so
top-k selections match the fp32 reference; W_o runs in fp32r with a 1/32 host
prescale that folds out the fp8 K/V/Q scale.  Attention is two-pass; pass A
fuses the softmax denominators (ones-matmul, software-pipelined one step
behind the score matmuls) plus reciprocal+broadcast per head, so pass B is a
dense run of AV matmuls that keeps the PE clock warm.  Causality comes from a
host-provided additive mask so the SPMD program is identical on every core.

PSUM accumulators that pack multiple regions per bank (attention pss/pot) are
pre-zeroed with memset and use start=False matmuls; full-bank restore
accumulators use start/stop directly.
"""
import sys

sys.path.insert(0, "/opt/trn_rl_repo")
import numpy as np
import concourse.bass as bass
import concourse.bacc as bacc
import concourse.mybir as mybir
import concourse.tile as tile
from concourse.bass_utils import run_bass_kernel_spmd
from concourse.masks import make_identity

F32 = mybir.dt.float32
F32R = mybir.dt.float32r
BF16 = mybir.dt.bfloat16
F8 = mybir.dt.float8e4
DR = mybir.MatmulPerfMode.DoubleRow
AX = mybir.AxisListType.X
OP = mybir.AluOpType
ACT = mybir.ActivationFunctionType

NCORES = 8
B, S, D, R, N, DS, TOPK, H = 2, 1024, 1024, 512, 32, 64, 4, 16
T = B * S
TL = T // NCORES          # tokens per core (256)
MT = TL // 128            # token tiles per core (2)
KT = D // 128             # contraction tiles over D (8)
DH = D // H               # head dim (64)
NRT = (N * R) // 128      # contraction tiles over N*R (128)
NPAIR = NRT // 2          # DoubleRow kt-pairs (64)
SEQ_BLOCKS = S // 128     # k blocks per sequence (8)
NEG = -1.0e30

# power-of-two fp8 scale plan (all rescales exact):
S_NX = 16.0               # nx, nx2 quantization scale
S_W = 512.0               # f_* and r_* pool host prescale
WB_SCALE = 1.0 / 128.0    # folded into transposed restore-route weights
PSUM_SCALE = 1.0 / 32768.0  # restore PSUM holds 32768*y
S_KV = 32.0               # fp8 scale of Q/K/V entering attention
STAGE_SCALE = S_KV * PSUM_SCALE     # 2^-10: restore PSUM -> 32*y
EXP_SCALE = 0.125 / (S_KV * S_KV)   # scores PSUM holds 1024*(QK)
WO_PRE = 1.0 / S_KV       # host prescale of W_o (ot carries 32*attnout)

# emb segment used by each of the 6 attention routings (fq, fk, fv, rq, rk, rv)
ATTN_SEG = [0, 0, 1, 2, 2, 3]

_PROG = None


def _routing(nc, rt, psmm, name, lhsT_ap, e_ap, wout):
    """top-4 sparsified + renormalized softmax over 32 logits -> wout
    [128,32] f32.  The dense-softmax denominator cancels in the
    renormalize (the reference's +1e-8 is ~2e-7 relative here, far below
    fp32 noise), and top-4 of softmax == top-4 of exp(logits - max), so
    selection runs directly on the exps.
    lhsT_ap: [64, 128] fp32 (h-segment transposed), e_ap: [64, 32] fp32."""
    lg = psmm.tile([128, N], F32, name=f"lg_{name}", tag="mm")
    nc.tensor.matmul(lg[:], lhsT_ap, e_ap, start=True, stop=True)
    mx = rt.tile([128, 1], F32, name=f"mx_{name}", tag="mx")
    nc.vector.tensor_reduce(mx[:], lg[:], AX, OP.max)
    nmx = rt.tile([128, 1], F32, name=f"nmx_{name}", tag="nmx")
    nc.scalar.mul(nmx[:], mx[:], -1.0)
    ex = rt.tile([128, N], F32, name=f"ex_{name}", tag="ex")
    nc.scalar.activation(ex[:], lg[:], ACT.Exp, bias=nmx[:], scale=1.0)
    top8 = rt.tile([128, 8], F32, name=f"top8_{name}", tag="top8")
    nc.vector.max(top8[:], ex[:])
    ge = rt.tile([128, N], F32, name=f"ge_{name}", tag="ge")
    nc.vector.tensor_scalar(ge[:], ex[:], top8[:, 3:4], None, OP.is_ge)
    sp = rt.tile([128, N], F32, name=f"sp_{name}", tag="sp")
    nc.vector.tensor_mul(sp[:], ex[:], ge[:])
    s2 = rt.tile([128, 1], F32, name=f"s2_{name}", tag="s2")
    nc.vector.tensor_reduce(s2[:], sp[:], AX, OP.add)
    rs2 = rt.tile([128, 1], F32, name=f"rs2_{name}", tag="rs2")
    nc.vector.reciprocal(rs2[:], s2[:])
    nc.vector.tensor_scalar_mul(wout, sp[:], rs2[:])


def _layernorm(nc, lnp, name, x_ap, s_bc, b_bc, out_ap):
    """LN over the free dim (D). x_ap/out_ap [128, D] f32; s_bc/b_bc [128, D]."""
    mu = lnp.tile([128, 1], F32, name=f"mu_{name}", tag="mu")
    nc.vector.tensor_reduce(mu[:], x_ap, AX, OP.add)
    nc.scalar.mul(mu[:], mu[:], 1.0 / D)
    xc = lnp.tile([128, D], F32, name=f"xc_{name}", tag="xc")
    nc.vector.tensor_scalar_sub(xc[:], x_ap, mu[:])
    sq = lnp.tile([128, D], F32, name=f"sq_{name}", tag="sq")
    vs = lnp.tile([128, 1], F32, name=f"vs_{name}", tag="vs")
    nc.scalar.activation(sq[:], xc[:], ACT.Square, accum_out=vs[:])
    nc.scalar.activation(vs[:], vs[:], ACT.Copy, scale=1.0 / D, bias=1e-6)
    rv = lnp.tile([128, 1], F32, name=f"rv_{name}", tag="rv")
    nc.vector.reciprocal(rv[:], vs[:])
    rstd = lnp.tile([128, 1], F32, name=f"rstd_{name}", tag="rstd")
    nc.scalar.activation(rstd[:], rv[:], ACT.Sqrt)
    nc.vector.scalar_tensor_tensor(out_ap, xc[:], rstd[:], s_bc, OP.mult, OP.mult)
    nc.vector.tensor_add(out_ap, out_ap, b_bc)


def _tr(nc, pstr, name, src_ap, ident, outs):
    """PE-transpose a [128, <=128] block; copy the psum into each
    (ap, engine, scale) — scale None for a plain copy."""
    p = pstr.tile([src_ap.shape[-1], 128], src_ap.dtype, name=f"tr_{name}",
                  tag="tr")
    nc.tensor.transpose(p[:], src_ap, ident)
    for ap, eng, scale in outs:
        if scale is None:
            if eng == "v":
                nc.vector.tensor_copy(ap, p[:, :ap.shape[-1]])
            else:
                nc.scalar.copy(ap, p[:, :ap.shape[-1]])
        else:
            if eng == "v":
                nc.vector.tensor_scalar_mul(ap, p[:, :ap.shape[-1]], scale)
            else:
                nc.scalar.activation(ap, p[:, :ap.shape[-1]], ACT.Copy,
                                     scale=scale)


def _feature(nc, fp, psf, fdram, nxT_r, routes, engs):
    """h[m] accumulators += w[:,n] * (nx @ f_n) for all 32 neurons, fp8
    DoubleRow over adjacent k-tile pairs.  F streams in half-neuron chunks
    (2 pairs each).  routes: list of (w_tiles_per_m, hacc_per_m)."""
    NP = KT // 2
    for n in range(N):
        pfs = [psf.tile([128, R], F32, name=f"pf{m}", tag=f"pf{m}")
               for m in range(MT)]
        for half in range(2):
            fc = fp.tile([128, 2, 2, R], F8, name="fc", tag="fc")
            engs[(2 * n + half) % len(engs)].dma_start(
                fc[:].rearrange("p a b r -> p (a b r)"),
                fdram[:, n, half * (KT // 2) * R:(half + 1) * (KT // 2) * R])
            for m in range(MT):
                for pi in range(2):
                    p = half * 2 + pi
                    nc.tensor.matmul(pfs[m][:],
                                     nxT_r[:, 2 * p:2 * p + 2,
                                           m * 128:(m + 1) * 128],
                                     fc[:, pi, :, :],
                                     start=(p == 0), stop=(p == NP - 1),
                                     perf_mode=DR)
        two = len(routes) == 2
        for m in range(MT):
            for ri, (wt, hacc) in enumerate(routes):
                # PSUM is DVE/ACT-only territory.  Spread the combine over
                # three engines: one path is a direct DVE STT from PSUM; the
                # other bounces PSUM->SBUF through an ACT copy so the
                # (SBUF-only) gpsimd engine can do the scaled accumulate.
                w_ap = wt[m][:, n:n + 1]
                direct = (ri == 0) if two else (m == 0)
                if direct:
                    if n == 0:
                        nc.vector.tensor_scalar(hacc[m][:], pfs[m][:], w_ap,
                                                None, OP.mult)
                    else:
                        nc.vector.scalar_tensor_tensor(hacc[m][:], pfs[m][:],
                                                       w_ap, hacc[m][:],
                                                       OP.mult, OP.add)
                else:
                    if n == 0:
                        nc.scalar.activation(hacc[m][:], pfs[m][:], ACT.Copy,
                                             scale=w_ap)
                    else:
                        # gpsimd supports plain tensor_tensor but not the
                        # [p,1]-scalar ops, so ACT applies the weight
                        tmp = fp.tile([128, R], BF16, name=f"cmb{m}",
                                      tag=f"cmb{m}")
                        nc.scalar.activation(tmp[:], pfs[m][:], ACT.Copy,
                                             scale=w_ap)
                        nc.gpsimd.tensor_add(hacc[m][:], hacc[m][:], tmp[:])


def _wb_prefetch(nc, wbp, wtd, key):
    """Broadcast the full [N, TL] restore-route weight table into SBUF with
    ONE fire-and-forget DMA on the gpsimd queue, issued BEFORE any collective
    lands there — a collective blocks the queue, and dozens of per-neuron
    broadcast triggers (~0.8us queue time each) starve whatever follows."""
    wb = wbp.tile([128, N, TL], BF16, name=f"wball_{key}")
    nc.gpsimd.dma_start(wb[:], wtd[0:1, :, :].broadcast_to([128, N, TL]))
    return wb


def _restore_tok(nc, rp, gtp, rdram, routes, psy_tiles, engs, dr=True,
                 gps_ok=True):
    """Token-major restores sharing one streamed r matrix.  dr=True: fp8
    DoubleRow over adjacent kt pairs; dr=False: bf16 (the V path — fp8 r_v
    quantization is token-correlated through attention averaging and flips
    knowledge routings).  routes: list of (hT [128,4,TL] bf16, wbs: 32
    prefetched [128,TL] bf16 weight-row tiles).  psy_tiles[ri][m][db]:
    full-bank [128,512] f32 accs.  gps_ok=False when a collective was just
    issued on the gpsimd queue (ops there would stall behind it)."""
    gdt = F8 if dr else BF16
    meng = nc.gpsimd if gps_ok else nc.vector
    for pair in range(NPAIR):
        n, sub = pair // 2, pair % 2
        rc = rp.tile([128, 2, D], rdram.dtype, name="rc", tag="rc")
        engs[pair % len(engs)].dma_start(rc[:], rdram[:, 2 * pair:2 * pair + 2, :])
        for ri, (hT, wball) in enumerate(routes):
            gt = gtp.tile([128, 2, TL], gdt, name=f"gt{ri}", tag=f"gt{ri}")
            nc.vector.tensor_mul(gt[:, 0, :], hT[:, 2 * sub, :],
                                 wball[:, n, :])
            meng.tensor_mul(gt[:, 1, :], hT[:, 2 * sub + 1, :],
                            wball[:, n, :])
            if dr:
                for m in range(MT):
                    for db in range(2):
                        nc.tensor.matmul(psy_tiles[ri][m][db][:],
                                         gt[:, :, m * 128:(m + 1) * 128],
                                         rc[:, :, db * 512:(db + 1) * 512],
                                         start=(pair == 0),
                                         stop=(pair == NPAIR - 1),
                                         perf_mode=DR)
            else:
                for j in range(2):
                    for m in range(MT):
                        for db in range(2):
                            nc.tensor.matmul(
                                psy_tiles[ri][m][db][:],
                                gt[:, j, m * 128:(m + 1) * 128],
                                rc[:, j, db * 512:(db + 1) * 512],
                                start=(pair == 0 and j == 0),
                                stop=(pair == NPAIR - 1 and j == 1))


def build(dbg=False):
    nc = bacc.Bacc("TRN2", target_bir_lowering=False, debug=False,
                   num_devices=NCORES)

    x_d = nc.dram_tensor("x", [TL, D], F32, kind="ExternalInput")
    maskT_d = nc.dram_tensor("maskT", [S, TL], F32, kind="ExternalInput")
    wall_d = nc.dram_tensor("wall", [128, KT, 6 * DS], F32, kind="ExternalInput")
    wo_d = nc.dram_tensor("wo", [128, KT, D], F32R, kind="ExternalInput")
    wfk_d = nc.dram_tensor("wfk", [128, KT, DS], F32, kind="ExternalInput")
    wrk_d = nc.dram_tensor("wrk", [128, KT, DS], F32, kind="ExternalInput")
    et_d = nc.dram_tensor("et", [DS, 6 * N], F32, kind="ExternalInput")
    fqk_d = nc.dram_tensor("fqk", [128, N, KT * R], F8, kind="ExternalInput")
    fv_d = nc.dram_tensor("fv", [128, N, KT * R], F8, kind="ExternalInput")
    fkn_d = nc.dram_tensor("fkn", [128, N, KT * R], F8, kind="ExternalInput")
    rqk_d = nc.dram_tensor("rqk", [128, NRT, D], F8, kind="ExternalInput")
    rv_d = nc.dram_tensor("rv", [128, NRT, D], BF16, kind="ExternalInput")
    rkn_d = nc.dram_tensor("rkn", [128, NRT, D], F8, kind="ExternalInput")
    ln_d = nc.dram_tensor("lnrows", [4, D], F32, kind="ExternalInput")
    bias_d = nc.dram_tensor("biasrow", [1, 8 * DS], F32, kind="ExternalInput")
    y_d = nc.dram_tensor("y", [TL, D], F32, kind="ExternalOutput")

    dbg_t = {}

    def dbg_tensor(name, shape):
        dbg_t[name] = nc.dram_tensor("dbg_" + name, shape, F32,
                                     kind="ExternalOutput")
        return dbg_t[name]

    with tile.TileContext(nc) as tc:
        with (
            tc.tile_pool(name="perm", bufs=1) as perm,
            tc.tile_pool(name="dramp", bufs=1, space="DRAM") as dramp,
            tc.tile_pool(name="lnp", bufs=1) as lnp,
            tc.tile_pool(name="rtp", bufs=2) as rtp,
        ):
            # collective bounce buffers (K^T gathered first, then token-major V)
            cck_in = dramp.tile([128, KT * TL], F8, name="cck_in")
            cck_out = dramp.tile([4 * 128, KT * TL], F8, name="cck_out")
            ccv_in = dramp.tile([128, MT * D], F8, name="ccv_in")
            ccv_out = dramp.tile([4 * 128, MT * D], F8, name="ccv_out")
            # restore-route w rows, bounced through DRAM into one partition
            wt_dram = {k: dramp.tile([1, N, TL], BF16, name=f"wtd_{k}")
                       for k in ("rq", "rk", "rv", "rkn")}

            ident = perm.tile([128, 128], F32)
            make_identity(nc, ident[:])
            ident_b = perm.tile([128, 128], BF16)
            nc.vector.tensor_copy(ident_b[:], ident[:])
            ones_f = perm.tile([128, 1], F32)
            nc.gpsimd.memset(ones_f[:], 1.0)
            ones_b = perm.tile([128, 1], BF16)
            nc.vector.tensor_copy(ones_b[:], ones_f[:])
            psc = perm.tile([128, 1], F32)
            nc.gpsimd.memset(psc[:], PSUM_SCALE)
            bias_bc = perm.tile([128, 8 * DS], F32)
            nc.sync.dma_start(bias_bc[:], bias_d[0:1, :].broadcast_to([128, 8 * DS]))
            et_sb = perm.tile([DS, 6 * N], F32)
            nc.sync.dma_start(et_sb[:], et_d[:])
            # copy at partition base 64 for routings whose h-segment sits in
            # the upper half of a transposed tile (matmul requires equal bases)
            et_hi = perm.tile([128, 6 * N], F32)
            nc.sync.dma_start(et_hi[DS:2 * DS, :], et_d[:])
            x_sb = perm.tile([128, MT, D], F32)
            for m in range(MT):
                nc.sync.dma_start(x_sb[:, m, :], x_d[m * 128:(m + 1) * 128, :])
            maskT_sb = perm.tile([128, SEQ_BLOCKS, TL], F32)
            yT_q = perm.tile([128, KT, TL], F8)

            # ============ stage 1: LN1 + routing + features + restores ========
            with (
                tc.tile_pool(name="st1", bufs=1) as st1,
                tc.tile_pool(name="fchunk", bufs=4) as fp,
                tc.tile_pool(name="rchunk", bufs=4) as rp,
                tc.tile_pool(name="gtp", bufs=3) as gtp,
            ):
                nxT_r = st1.tile([128, KT, TL], F8)
                h_q = [st1.tile([128, R], BF16, name=f"h_q{m}") for m in range(MT)]
                h_k = [st1.tile([128, R], BF16, name=f"h_k{m}") for m in range(MT)]
                h_v = [st1.tile([128, R], BF16, name=f"h_v{m}") for m in range(MT)]
                hT = {k: st1.tile([128, 4, TL], BF16, name=f"hT_{k}")
                      for k in ("q", "k", "v")}
                w_feat = {p: [st1.tile([128, N], F32, name=f"w{p}_{m}")
                              for m in range(MT)] for p in range(3)}
                wtt_sb = {k: st1.tile([N, TL], BF16, name=f"wtt_{k}")
                          for k in ("rq", "rk", "rv")}
                yT_k = st1.tile([128, KT, TL], F8)
                kst = st1.tile([128, MT, D], BF16, name="kst")
                qst = st1.tile([128, MT, D], BF16, name="qst")
                v_tok = st1.tile([128, MT, D], F8)

                with (
                    tc.tile_pool(name="st1a", bufs=1) as st1a,
                    tc.tile_pool(name="ps_tr", bufs=2, space="PSUM") as pstr,
                    tc.tile_pool(name="ps_mm", bufs=2, space="PSUM") as psmm,
                    tc.tile_pool(name="ps_feat", bufs=2, space="PSUM") as psf,
                    tc.tile_pool(name="wallp", bufs=2) as wallp,
                ):
                    nxT = st1a.tile([128, KT, TL], F32)
                    nx = st1a.tile([128, MT, D], F32)
                    ln1_bc = st1a.tile([128, 2, D], F32)
                    for i in range(2):
                        nc.gpsimd.dma_start(ln1_bc[:, i, :],
                                            ln_d[i:i + 1, :]
                                            .broadcast_to([128, D]))
                    for m in range(MT):
                        _layernorm(nc, lnp, f"ln1_{m}", x_sb[:, m, :],
                                   ln1_bc[:, 0, :], ln1_bc[:, 1, :], nx[:, m, :])
                    for m in range(MT):
                        for k in range(KT):
                            _tr(nc, pstr, f"nx_{m}_{k}",
                                nx[:, m, k * 128:(k + 1) * 128], ident[:],
                                [(nxT[:, k, m * 128:(m + 1) * 128], "v", None),
                                 (nxT_r[:, k, m * 128:(m + 1) * 128], "s",
                                  S_NX)])

                    hall = st1a.tile([128, MT, 6 * DS], F32)
                    phs = [psmm.tile([128, 6 * DS], F32, name=f"ph{m}",
                                     tag="mm") for m in range(MT)]
                    for k in range(KT):
                        wt_k = wallp.tile([128, 6 * DS], F32, name="wal",
                                          tag="wal")
                        [nc.sync, nc.scalar][k % 2].dma_start(
                            wt_k[:], wall_d[:, k, :])
                        for m in range(MT):
                            nc.tensor.matmul(phs[m][:],
                                             nxT[:, k, m * 128:(m + 1) * 128],
                                             wt_k[:],
                                             start=(k == 0), stop=(k == KT - 1))
                    for m in range(MT):
                        nc.vector.tensor_add(hall[:, m, :], phs[m][:],
                                             bias_bc[:, :6 * DS])
                    # prefetch the attention mask well before stage 2
                    for kb in range(SEQ_BLOCKS):
                        nc.gpsimd.dma_start(maskT_sb[:, kb, :],
                                            maskT_d[kb * 128:(kb + 1) * 128, :])
                    hallT = st1a.tile([128, 3, TL], F32)
                    for m in range(MT):
                        for i in range(3):
                            _tr(nc, pstr, f"ha_{m}_{i}",
                                hall[:, m, i * 128:(i + 1) * 128], ident[:],
                                [(hallT[:, i, m * 128:(m + 1) * 128], "v",
                                  None)])
                    w_rest = {}
                    for p in range(6):
                        seg = ATTN_SEG[p]
                        tiles = w_feat[p] if p < 3 else \
                            [st1a.tile([128, N], F32, name=f"w{p}_{m}")
                             for m in range(MT)]
                        if p >= 3:
                            w_rest[p] = tiles
                        for m in range(MT):
                            base, ti = (p % 2) * DS, p // 2
                            e_src = et_sb if base == 0 else et_hi
                            e_ap = e_src[base:base + DS,
                                         seg * N:(seg + 1) * N]
                            _routing(nc, rtp, psmm, f"r{p}_{m}",
                                     hallT[base:base + DS, ti,
                                           m * 128:(m + 1) * 128],
                                     e_ap, tiles[m][:])
                    wbsets = {}
                    for p, key in [(3, "rq"), (4, "rk"), (5, "rv")]:
                        for m in range(MT):
                            _tr(nc, pstr, f"wt_{p}_{m}", w_rest[p][m][:],
                                ident[:],
                                [(wtt_sb[key][:, m * 128:(m + 1) * 128], "v",
                                  WB_SCALE)])
                        nc.gpsimd.dma_start(wt_dram[key][0], wtt_sb[key][:])
                    # all 96 weight-row broadcasts issued here, before any
                    # collective lands on the gpsimd queue
                    for key in ("rk", "rv", "rq"):
                        wbsets[key] = _wb_prefetch(nc, st1, wt_dram[key], key)

                    # features (qk shared for Q and K; v)
                    _feature(nc, fp, psf, fqk_d, nxT_r,
                             [(w_feat[0], h_q), (w_feat[1], h_k)],
                             [nc.sync, nc.scalar])
                    _feature(nc, fp, psf, fv_d, nxT_r,
                             [(w_feat[2], h_v)], [nc.sync, nc.scalar])
                    for nm, hh in [("q", h_q), ("k", h_k), ("v", h_v)]:
                        for m in range(MT):
                            for rb in range(4):
                                _tr(nc, pstr, f"h{nm}_{m}_{rb}",
                                    hh[m][:, rb * 128:(rb + 1) * 128], ident_b[:],
                                    [(hT[nm][:, rb, m * 128:(m + 1) * 128],
                                      "v", None)])

                # restores, split so each collective hides under the next
                # restore's compute: K -> AG(K) -> V -> AG(V) -> Q.  r_qk
                # streams twice (fp8, cheap) to buy the earlier K gather.
                def _stage_out(ps_tiles, st, scale):
                    for m in range(MT):
                        for db in range(2):
                            dst = st[:, m, db * 512:(db + 1) * 512]
                            if (m + db) % 2 == 0:
                                nc.scalar.activation(dst, ps_tiles[m][db][:],
                                                     ACT.Copy, scale=scale)
                            else:
                                nc.vector.tensor_scalar_mul(
                                    dst, ps_tiles[m][db][:], scale)

                with tc.tile_pool(name="ps_yk", bufs=1, space="PSUM") as psy:
                    pks = [[psy.tile([128, 512], F32, name=f"pk{m}{db}")
                            for db in range(2)] for m in range(MT)]
                    _restore_tok(nc, rp, gtp, rqk_d,
                                 [(hT["k"][:], wbsets["rk"])],
                                 [pks], [nc.sync, nc.scalar], gps_ok=False)
                    _stage_out(pks, kst, STAGE_SCALE)
                with tc.tile_pool(name="ps_tr2", bufs=2, space="PSUM") as pstr2:
                    for m in range(MT):
                        for dt in range(KT):
                            _tr(nc, pstr2, f"kT_{m}_{dt}",
                                kst[:, m, dt * 128:(dt + 1) * 128], ident_b[:],
                                [(yT_k[:, dt, m * 128:(m + 1) * 128], "v",
                                  None)])
                    # K^T gather starts while the V + Q restores compute
                    nc.sync.dma_start(cck_in[:],
                                      yT_k[:].rearrange("p k t -> p (k t)"))
                    nc.gpsimd.collective_compute(
                        "AllGather", OP.bypass,
                        ins=[cck_in[:]],
                        outs=[cck_out[:]],
                        replica_groups=[[0, 1, 2, 3], [4, 5, 6, 7]],
                    )
                # V restore, token-major straight into its collective payload
                with tc.tile_pool(name="ps_yv", bufs=1, space="PSUM") as psy:
                    pvs = [[psy.tile([128, 512], F32, name=f"pv{m}{db}")
                            for db in range(2)] for m in range(MT)]
                    _restore_tok(nc, rp, gtp, rv_d,
                                 [(hT["v"][:], wbsets["rv"])],
                                 [pvs], [nc.sync, nc.scalar], dr=False,
                                 gps_ok=False)
                    _stage_out(pvs, v_tok, STAGE_SCALE)
                for m in range(MT):
                    nc.sync.dma_start(ccv_in[:, m * D:(m + 1) * D],
                                      v_tok[:, m, :])
                nc.gpsimd.collective_compute(
                    "AllGather", OP.bypass,
                    ins=[ccv_in[:]],
                    outs=[ccv_out[:]],
                    replica_groups=[[0, 1, 2, 3], [4, 5, 6, 7]],
                )
                with tc.tile_pool(name="ps_yq", bufs=1, space="PSUM") as psy:
                    pqs = [[psy.tile([128, 512], F32, name=f"pq{m}{db}")
                            for db in range(2)] for m in range(MT)]
                    _restore_tok(nc, rp, gtp, rqk_d,
                                 [(hT["q"][:], wbsets["rq"])],
                                 [pqs], [nc.sync, nc.scalar], gps_ok=False)
                    _stage_out(pqs, qst, STAGE_SCALE)
                with tc.tile_pool(name="ps_tr2b", bufs=2, space="PSUM") as pstr2:
                    for m in range(MT):
                        for dt in range(KT):
                            _tr(nc, pstr2, f"qT_{m}_{dt}",
                                qst[:, m, dt * 128:(dt + 1) * 128], ident_b[:],
                                [(yT_q[:, dt, m * 128:(m + 1) * 128], "s",
                                  None)])

            # ============ stage 2: attention + W_o ============
            late_cm = tc.tile_pool(name="late", bufs=1)
            late = late_cm.__enter__()
            x2 = late.tile([128, MT, D], F32)
            ot_sb = late.tile([128, KT, TL], F32)
            with (
                tc.tile_pool(name="st2", bufs=1) as st2,
                tc.tile_pool(name="attp", bufs=3) as att,
                tc.tile_pool(name="ps_att", bufs=2, space="PSUM") as psa,
                tc.tile_pool(name="ps_ot", bufs=4, space="PSUM") as psot,
            ):
                # phase A: scores + exp + softmax denominators for all heads —
                # the V AllGather and v_all loads hide under this pass.  Exps
                # run 4 k-blocks per ACT instruction; denominator matmuls lag
                # one quarter so the PE never waits on the exp chain; all 16
                # reciprocals batch into one broadcast + one DVE op.
                expt_all = st2.tile([128, 2 * KT, SEQ_BLOCKS, TL], BF16)
                rbc_all = st2.tile([DH, H, TL], F32)
                sums_sb = st2.tile([1, H, TL], F32)
                with tc.tile_pool(name="ktp", bufs=1) as ktp:
                    kt_all = ktp.tile([128, 4, KT * TL], F8)
                    # not gpsimd: its queue is still draining the V gather
                    ld_engs = [nc.sync, nc.scalar]
                    for ch in range(4):
                        ld_engs[ch % 2].dma_start(
                            kt_all[:, ch, :],
                            cck_out[ch * 128:(ch + 1) * 128, :])
                    for hp in range(KT):
                        for hh in range(2):
                            h_idx = hp * 2 + hh
                            qt_ap = yT_q[hh * DH:(hh + 1) * DH, hp, :]
                            pss = psa.tile([1, TL], F32, name="pss",
                                           tag="pss")
                            nc.vector.memset(pss[:], 0.0)
                            for kbq in range(2):
                                msc4 = att.tile([128, 4, TL], F32,
                                                name="msc4", tag="msc")
                                for j in range(4):
                                    kb = kbq * 4 + j
                                    ch, m2 = kb // 2, kb % 2
                                    ktap = kt_all[hh * DH:(hh + 1) * DH, ch,
                                                  hp * TL + m2 * 128:
                                                  hp * TL + (m2 + 1) * 128]
                                    pscore = psa.tile([128, TL], F32,
                                                      name="pscore",
                                                      tag="pscore")
                                    nc.tensor.matmul(pscore[:], ktap, qt_ap,
                                                     start=True, stop=True)
                                    # spread the mask adds: DVE direct from
                                    # PSUM, or ACT copy-out + gpsimd add
                                    if kb % 2 == 0:
                                        nc.vector.tensor_add(
                                            msc4[:, j, :], pscore[:],
                                            maskT_sb[:, kb, :])
                                    else:
                                        mtmp = att.tile([128, TL], F32,
                                                        name="mtmp",
                                                        tag="mtmp")
                                        nc.scalar.copy(mtmp[:], pscore[:])
                                        nc.gpsimd.tensor_add(
                                            msc4[:, j, :], mtmp[:],
                                            maskT_sb[:, kb, :])
                                nc.scalar.activation(
                                    expt_all[:, h_idx, 4 * kbq:4 * kbq + 4, :],
                                    msc4[:], ACT.Exp, scale=EXP_SCALE)
                                if kbq == 1:
                                    for kb in range(4):
                                        nc.tensor.matmul(
                                            pss[:], ones_b[:],
                                            expt_all[:, h_idx, kb, :],
                                            start=False, stop=False)
                            for kb in range(4, SEQ_BLOCKS):
                                nc.tensor.matmul(
                                    pss[:], ones_b[:],
                                    expt_all[:, h_idx, kb, :],
                                    start=False, stop=(kb == SEQ_BLOCKS - 1))
                            nc.vector.tensor_copy(sums_sb[0:1, h_idx, :],
                                                  pss[:])
                    # DVE reciprocal costs ~15 cyc/element on ONE partition's
                    # free dim, so respread all 16 heads' sums across 128
                    # partitions via DMA, reciprocate there (~30 elem each),
                    # and DMA back before the per-head broadcasts.
                    spr = st2.tile([128, H * TL // 128], F32)
                    spr_r = st2.tile([128, H * TL // 128], F32)
                    rinv = st2.tile([1, H, TL], F32)
                    nc.sync.dma_start(
                        spr[:], sums_sb[:].rearrange("p h t -> p (h t)"))
                    nc.vector.reciprocal(spr_r[:], spr[:])
                    nc.sync.dma_start(
                        rinv[:].rearrange("p h t -> p (h t)"), spr_r[:])
                    for h_idx in range(H):
                        nc.gpsimd.partition_broadcast(
                            rbc_all[:, h_idx, :], rinv[0:1, h_idx, :],
                            channels=DH)
                # phase B: AV — a dense run of matmuls
                v_all = st2.tile([128, 4, MT * D], F8)
                ld_engs = [nc.sync, nc.scalar, nc.gpsimd]
                for ch in range(4):
                    ld_engs[ch % 3].dma_start(
                        v_all[:, ch, :], ccv_out[ch * 128:(ch + 1) * 128, :])
                for hp in range(KT):
                    for hh in range(2):
                        h_idx = hp * 2 + hh
                        pot = psot.tile([DH, TL], F32, name="pot", tag="pot")
                        nc.vector.memset(pot[:], 0.0)
                        for kb in range(SEQ_BLOCKS):
                            ch, m2 = kb // 2, kb % 2
                            vap = v_all[:, ch,
                                        m2 * D + h_idx * DH:
                                        m2 * D + (h_idx + 1) * DH]
                            nc.tensor.matmul(pot[:], vap,
                                             expt_all[:, h_idx, kb, :],
                                             start=False,
                                             stop=(kb == SEQ_BLOCKS - 1))
                        otn = att.tile([DH, TL], F32, name="otn", tag="otn")
                        nc.vector.tensor_mul(otn[:], pot[:],
                                             rbc_all[:, h_idx, :])
                        # SBUF->SBUF DMA can shift partitions (DVE cannot)
                        nc.sync.dma_start(ot_sb[hh * DH:(hh + 1) * DH, hp, :],
                                          otn[:])

            with (
                tc.tile_pool(name="wop", bufs=3) as wop,
                tc.tile_pool(name="ps_mm2", bufs=2, space="PSUM") as psmm2,
            ):
                ot_r = ot_sb[:].bitcast(F32R)
                for blk in range(2):
                    wo_t = []
                    for k in range(KT):
                        wt_k = wop.tile([128, 512], F32R, name=f"wo{k}",
                                        tag="wo")
                        [nc.sync, nc.scalar][k % 2].dma_start(
                            wt_k[:], wo_d[:, k, blk * 512:(blk + 1) * 512])
                        wo_t.append(wt_k)
                    for m in range(MT):
                        px = psmm2.tile([128, 512], F32, name="px", tag="mm")
                        for k in range(KT):
                            nc.tensor.matmul(px[:],
                                             ot_r[:, k, m * 128:(m + 1) * 128],
                                             wo_t[k][:],
                                             start=(k == 0), stop=(k == KT - 1))
                        nc.vector.tensor_add(
                            x2[:, m, blk * 512:(blk + 1) * 512], px[:],
                            x_sb[:, m, blk * 512:(blk + 1) * 512])
            if dbg:
                td = dbg_tensor("x2", [TL, D])
                for m in range(MT):
                    nc.sync.dma_start(td[m * 128:(m + 1) * 128, :], x2[:, m, :])

            # ============ stage 3: knowledge circuit ============
            with (
                tc.tile_pool(name="st3", bufs=1) as st3,
                tc.tile_pool(name="fchunk2", bufs=3) as fp2,
                tc.tile_pool(name="rchunk2", bufs=3) as rp2,
                tc.tile_pool(name="gtp2", bufs=3) as gtp2,
            ):
                nx2T_r = st3.tile([128, KT, TL], F8)
                h_kn = [st3.tile([128, R], BF16, name=f"h_kn{m}")
                        for m in range(MT)]
                hT_kn = st3.tile([128, 4, TL], BF16)
                wtt_kn = st3.tile([N, TL], BF16, name="wtt_kn")
                w_kn = {}
                with (
                    tc.tile_pool(name="st3a", bufs=1) as st3a,
                    tc.tile_pool(name="ps_tr3", bufs=2, space="PSUM") as pstr3,
                    tc.tile_pool(name="ps_mm3", bufs=2, space="PSUM") as psmm3,
                    tc.tile_pool(name="ps_feat3", bufs=2, space="PSUM") as psf3,
                ):
                    nx2 = st3a.tile([128, MT, D], F32)
                    ln2_bc = st3a.tile([128, 2, D], F32)
                    for i in range(2):
                        nc.sync.dma_start(
                            ln2_bc[:, i, :],
                            ln_d[i + 2:i + 3, :].broadcast_to([128, D]))
                    for m in range(MT):
                        _layernorm(nc, lnp, f"ln2_{m}", x2[:, m, :],
                                   ln2_bc[:, 0, :], ln2_bc[:, 1, :], nx2[:, m, :])
                    nx2T = st3a.tile([128, KT, TL], F32)
                    for m in range(MT):
                        for k in range(KT):
                            _tr(nc, pstr3, f"nx2_{m}_{k}",
                                nx2[:, m, k * 128:(k + 1) * 128], ident[:],
                                [(nx2T[:, k, m * 128:(m + 1) * 128], "v", None),
                                 (nx2T_r[:, k, m * 128:(m + 1) * 128], "s",
                                  S_NX)])
                    wk_sb = st3a.tile([128, KT, 2 * DS], F32)
                    nc.sync.dma_start(wk_sb[:, :, :DS], wfk_d[:])
                    nc.sync.dma_start(wk_sb[:, :, DS:], wrk_d[:])
                    hkT = st3a.tile([DS, 2, TL], F32)
                    for m in range(MT):
                        for j in range(2):
                            pk = psmm3.tile([128, DS], F32, name="pk", tag="mm")
                            for k in range(KT):
                                nc.tensor.matmul(
                                    pk[:], nx2T[:, k, m * 128:(m + 1) * 128],
                                    wk_sb[:, k, j * DS:(j + 1) * DS],
                                    start=(k == 0), stop=(k == KT - 1))
                            hk = rtp.tile([128, DS], F32, name=f"hk{m}{j}",
                                          tag="hk")
                            nc.vector.tensor_add(
                                hk[:], pk[:],
                                bias_bc[:, (6 + j) * DS:(7 + j) * DS])
                            _tr(nc, pstr3, f"hk_{m}_{j}", hk[:], ident[:],
                                [(hkT[:, j, m * 128:(m + 1) * 128], "v",
                                  None)])
                    for j, nm in [(0, "fkn"), (1, "rkn")]:
                        w_kn[nm] = []
                        for m in range(MT):
                            wt = st3.tile([128, N], F32, name=f"wkn{j}_{m}")
                            _routing(nc, rtp, psmm3, f"rk{j}_{m}",
                                     hkT[:, j, m * 128:(m + 1) * 128],
                                     et_sb[:, (4 + j) * N:(5 + j) * N], wt[:])
                            w_kn[nm].append(wt)
                    for m in range(MT):
                        _tr(nc, pstr3, f"wt_kn_{m}", w_kn["rkn"][m][:],
                            ident[:],
                            [(wtt_kn[:, m * 128:(m + 1) * 128], "v",
                              WB_SCALE)])
                    nc.gpsimd.dma_start(wt_dram["rkn"][0], wtt_kn[:])
                    wbs_kn = _wb_prefetch(nc, st3, wt_dram["rkn"], "rkn")

                    _feature(nc, fp2, psf3, fkn_d, nx2T_r,
                             [(w_kn["fkn"], h_kn)],
                             [nc.sync, nc.scalar])
                    for m in range(MT):
                        for rb in range(4):
                            _tr(nc, pstr3, f"hkn_{m}_{rb}",
                                h_kn[m][:, rb * 128:(rb + 1) * 128], ident_b[:],
                                [(hT_kn[:, rb, m * 128:(m + 1) * 128], "v",
                                  None)])

                out_sb = st3.tile([128, MT, D], F32)
                with tc.tile_pool(name="ps_y3", bufs=1, space="PSUM") as psy3:
                    pkn = [[[psy3.tile([128, 512], F32, name=f"pn{m}{db}")
                             for db in range(2)] for m in range(MT)]]
                    _restore_tok(nc, rp2, gtp2, rkn_d,
                                 [(hT_kn[:], wbs_kn)],
                                 pkn, [nc.sync, nc.scalar])
                    for m in range(MT):
                        for db in range(2):
                            nc.vector.scalar_tensor_tensor(
                                out_sb[:, m, db * 512:(db + 1) * 512],
                                pkn[0][m][db][:], psc[:],
                                x2[:, m, db * 512:(db + 1) * 512],
                                OP.mult, OP.add)
                for m in range(MT):
                    nc.sync.dma_start(y_d[m * 128:(m + 1) * 128, :],
                                      out_sb[:, m, :])
            late_cm.__exit__(None, None, None)

    nc.compile()
    return nc, dbg_t


def prep_inputs(inputs):
    f32 = np.float32
    fp8 = mybir.dt.np(F8)
    x = np.ascontiguousarray(np.asarray(inputs["x"], f32).reshape(T, D))
    ne = np.asarray(inputs["neuron_emb"], f32)
    emb = ne / (np.linalg.norm(ne, axis=-1, keepdims=True) + 1e-8)

    def f_layout(f):
        f = np.asarray(f, f32) * S_W
        return np.ascontiguousarray(
            f.reshape(N, KT, 128, R).transpose(2, 0, 1, 3)
            .reshape(128, N, KT * R).astype(fp8))

    def r_layout(r, dt=None):
        r = np.asarray(r, f32).reshape(N * R, D) * S_W
        return np.ascontiguousarray(
            r.reshape(NRT, 128, D).transpose(1, 0, 2).astype(dt or fp8))

    def w_layout(w, pre=1.0):
        w = np.asarray(w, f32) * pre
        return np.ascontiguousarray(
            w.reshape(KT, 128, w.shape[-1]).transpose(1, 0, 2))

    shared = {
        "wall": w_layout(inputs["W_all"]),
        "wo": w_layout(inputs["W_o"], WO_PRE),
        "wfk": w_layout(inputs["W_fk"]),
        "wrk": w_layout(inputs["W_rk"]),
        "et": np.ascontiguousarray(emb.T),
        "fqk": f_layout(inputs["f_qk"]),
        "fv": f_layout(inputs["f_v"]),
        "fkn": f_layout(inputs["f_know"]),
        "rqk": r_layout(inputs["r_qk"]),
        "rv": r_layout(inputs["r_v"], mybir.dt.np(BF16)),
        "rkn": r_layout(inputs["r_know"]),
        "lnrows": np.ascontiguousarray(
            np.stack([np.asarray(inputs[k], f32)
                      for k in ("ln1_s", "ln1_b", "ln2_s", "ln2_b")])),
        "biasrow": np.ascontiguousarray(
            np.concatenate([np.asarray(inputs["b_all"], f32),
                            np.asarray(inputs["b_fk"], f32),
                            np.asarray(inputs["b_rk"], f32)])[None, :]),
    }
    per_core = []
    k_idx = np.arange(S)[:, None]
    for c in range(NCORES):
        ci = c % (S // TL)
        q_idx = ci * TL + np.arange(TL)[None, :]
        maskT = np.where(k_idx <= q_idx, 0.0, NEG).astype(f32)
        per_core.append({
            "x": np.ascontiguousarray(x[c * TL:(c + 1) * TL]),
            "maskT": np.ascontiguousarray(maskT),
            **shared,
        })
    return per_core


def kernel(**inputs):
    global _PROG
    if _PROG is None:
        _PROG = build(dbg=False)
    nc, _ = _PROG
    per_core = prep_inputs(inputs)
    res = run_bass_kernel_spmd(nc, per_core, core_ids=list(range(NCORES)))
    y = np.concatenate([res.results[c]["y"] for c in range(NCORES)], axis=0)
    return y.reshape(B, S, D).astype(np.float32)


# revision 43
# speedup vs baseline: 1.1156x; 1.1156x over previous
"""Trainium2 Bass kernel for the moe_routing problem (nn_DAWN_69904887709893).

Token-parallel across 8 NeuronCores (256 tokens/core), neuron pools replicated.
The six neuron pools (f_qk/f_v/f_know, r_qk/r_v/r_know) stream from HBM in
fp8 (e4m3, x512 host prescale) and all feature/restore matmuls run in fp8
DoubleRow perf mode (2 contraction rows per PE cell), pairing adjacent
128-row k-tiles.  All scale factors are powers of two so rescaling is exact:
nx is quantized x16, the restore route weights carry 1/128, and every restore
PSUM holds 32768*y, folded out in the PSUM->SBUF copy.  Restores are
token-major (stationary = fp8 token-side operand, moving = fp8 r chunk,
full-bank [128,512] accumulators with clean start/stop), so the knowledge
restore needs no final transposes; Q/K are transposed + quantized to fp8
after the fused Q+K restore, K^T is AllGathered in fp8 (half the wire bytes)
while the V restore computes, and V gathers token-major fp8.  Matmuls feeding
router logits (W_all, logit projections, W_fk/W_rk) stay in plain fp32 so
top-k selections match the fp32 reference; W_o runs in fp32r with a 1/32 host
prescale that folds out the fp8 K/V/Q scale.  Attention is two-pass; pass A
fuses the softmax denominators (ones-matmul, software-pipelined one step
behind the score matmuls) plus reciprocal+broadcast per head, so pass B is a
dense run of AV matmuls that keeps the PE clock warm.  Causality comes from a
host-provided additive mask so the SPMD program is identical on every core.

PSUM accumulators that pack multiple regions per bank (attention pss/pot) are
pre-zeroed with memset and use start=False matmuls; full-bank restore
accumulators use start/stop directly.
"""
import sys

sys.path.insert(0, "/opt/trn_rl_repo")
import numpy as np
import concourse.bass as bass
import concourse.bacc as bacc
import concourse.mybir as mybir
import concourse.tile as tile
from concourse.bass_utils import run_bass_kernel_spmd
from concourse.masks import make_identity

F32 = mybir.dt.float32
F32R = mybir.dt.float32r
BF16 = mybir.dt.bfloat16
F8 = mybir.dt.float8e4
DR = mybir.MatmulPerfMode.DoubleRow
AX = mybir.AxisListType.X
OP = mybir.AluOpType
ACT = mybir.ActivationFunctionType

NCORES = 8
B, S, D, R, N, DS, TOPK, H = 2, 1024, 1024, 512, 32, 64, 4, 16
T = B * S
TL = T // NCORES          # tokens per core (256)
MT = TL // 128            # token tiles per core (2)
KT = D // 128             # contraction tiles over D (8)
DH = D // H               # head dim (64)
NRT = (N * R) // 128      # contraction tiles over N*R (128)
NPAIR = NRT // 2          # DoubleRow kt-pairs (64)
SEQ_BLOCKS = S // 128     # k blocks per sequence (8)
NEG = -1.0e30

# power-of-two fp8 scale plan (all rescales exact):
S_NX = 16.0               # nx, nx2 quantization scale
S_W = 512.0               # f_* and r_* pool host prescale
WB_SCALE = 1.0 / 128.0    # folded into transposed restore-route weights
PSUM_SCALE = 1.0 / 32768.0  # restore PSUM holds 32768*y
S_KV = 32.0               # fp8 scale of Q/K/V entering attention
STAGE_SCALE = S_KV * PSUM_SCALE     # 2^-10: restore PSUM -> 32*y
EXP_SCALE = 0.125 / (S_KV * S_KV)   # scores PSUM holds 1024*(QK)
WO_PRE = 1.0 / S_KV       # host prescale of W_o (ot carries 32*attnout)

# emb segment used by each of the 6 attention routings (fq, fk, fv, rq, rk, rv)
ATTN_SEG = [0, 0, 1, 2, 2, 3]

_PROG = None


def _routing(nc, rt, psmm, name, lhsT_ap, e_ap, wout):
    """top-4 sparsified + renormalized softmax over 32 logits -> wout
    [128,32] f32.  The dense-softmax denominator cancels in the
    renormalize (the reference's +1e-8 is ~2e-7 relative here, far below
    fp32 noise), and top-4 of softmax == top-4 of exp(logits - max), so
    selection runs directly on the exps.
    lhsT_ap: [64, 128] fp32 (h-segment transposed), e_ap: [64, 32] fp32."""
    lg = psmm.tile([128, N], F32, name=f"lg_{name}", tag="mm")
    nc.tensor.matmul(lg[:], lhsT_ap, e_ap, start=True, stop=True)
    mx = rt.tile([128, 1], F32, name=f"mx_{name}", tag="mx")
    nc.vector.tensor_reduce(mx[:], lg[:], AX, OP.max)
    nmx = rt.tile([128, 1], F32, name=f"nmx_{name}", tag="nmx")
    nc.scalar.mul(nmx[:], mx[:], -1.0)
    ex = rt.tile([128, N], F32, name=f"ex_{name}", tag="ex")
    nc.scalar.activation(ex[:], lg[:], ACT.Exp, bias=nmx[:], scale=1.0)
    top8 = rt.tile([128, 8], F32, name=f"top8_{name}", tag="top8")
    nc.vector.max(top8[:], ex[:])
    ge = rt.tile([128, N], F32, name=f"ge_{name}", tag="ge")
    nc.vector.tensor_scalar(ge[:], ex[:], top8[:, 3:4], None, OP.is_ge)
    sp = rt.tile([128, N], F32, name=f"sp_{name}", tag="sp")
    nc.vector.tensor_mul(sp[:], ex[:], ge[:])
    s2 = rt.tile([128, 1], F32, name=f"s2_{name}", tag="s2")
    nc.vector.tensor_reduce(s2[:], sp[:], AX, OP.add)
    rs2 = rt.tile([128, 1], F32, name=f"rs2_{name}", tag="rs2")
    nc.vector.reciprocal(rs2[:], s2[:])
    nc.vector.tensor_scalar_mul(wout, sp[:], rs2[:])


def _layernorm(nc, lnp, name, x_ap, s_bc, b_bc, out_ap):
    """LN over the free dim (D). x_ap/out_ap [128, D] f32; s_bc/b_bc [128, D]."""
    mu = lnp.tile([128, 1], F32, name=f"mu_{name}", tag="mu")
    nc.vector.tensor_reduce(mu[:], x_ap, AX, OP.add)
    nc.scalar.mul(mu[:], mu[:], 1.0 / D)
    xc = lnp.tile([128, D], F32, name=f"xc_{name}", tag="xc")
    nc.vector.tensor_scalar_sub(xc[:], x_ap, mu[:])
    sq = lnp.tile([128, D], F32, name=f"sq_{name}", tag="sq")
    vs = lnp.tile([128, 1], F32, name=f"vs_{name}", tag="vs")
    nc.scalar.activation(sq[:], xc[:], ACT.Square, accum_out=vs[:])
    nc.scalar.activation(vs[:], vs[:], ACT.Copy, scale=1.0 / D, bias=1e-6)
    rv = lnp.tile([128, 1], F32, name=f"rv_{name}", tag="rv")
    nc.vector.reciprocal(rv[:], vs[:])
    rstd = lnp.tile([128, 1], F32, name=f"rstd_{name}", tag="rstd")
    nc.scalar.activation(rstd[:], rv[:], ACT.Sqrt)
    nc.vector.scalar_tensor_tensor(out_ap, xc[:], rstd[:], s_bc, OP.mult, OP.mult)
    nc.vector.tensor_add(out_ap, out_ap, b_bc)


def _tr(nc, pstr, name, src_ap, ident, outs):
    """PE-transpose a [128, <=128] block; copy the psum into each
    (ap, engine, scale) — scale None for a plain copy."""
    p = pstr.tile([src_ap.shape[-1], 128], src_ap.dtype, name=f"tr_{name}",
                  tag="tr")
    nc.tensor.transpose(p[:], src_ap, ident)
    for ap, eng, scale in outs:
        if scale is None:
            if eng == "v":
                nc.vector.tensor_copy(ap, p[:, :ap.shape[-1]])
            else:
                nc.scalar.copy(ap, p[:, :ap.shape[-1]])
        else:
            if eng == "v":
                nc.vector.tensor_scalar_mul(ap, p[:, :ap.shape[-1]], scale)
            else:
                nc.scalar.activation(ap, p[:, :ap.shape[-1]], ACT.Copy,
                                     scale=scale)


def _feature(nc, fp, psf, fdram, nxT_r, routes, engs):
    """h[m] accumulators += w[:,n] * (nx @ f_n) for all 32 neurons, fp8
    DoubleRow over adjacent k-tile pairs.  F streams in half-neuron chunks
    (2 pairs each).  routes: list of (w_tiles_per_m, hacc_per_m)."""
    NP = KT // 2
    for n in range(N):
        pfs = [psf.tile([128, R], F32, name=f"pf{m}", tag=f"pf{m}")
               for m in range(MT)]
        for half in range(2):
            fc = fp.tile([128, 2, 2, R], F8, name="fc", tag="fc")
            engs[(2 * n + half) % len(engs)].dma_start(
                fc[:].rearrange("p a b r -> p (a b r)"),
                fdram[:, n, half * (KT // 2) * R:(half + 1) * (KT // 2) * R])
            for m in range(MT):
                for pi in range(2):
                    p = half * 2 + pi
                    nc.tensor.matmul(pfs[m][:],
                                     nxT_r[:, 2 * p:2 * p + 2,
                                           m * 128:(m + 1) * 128],
                                     fc[:, pi, :, :],
                                     start=(p == 0), stop=(p == NP - 1),
                                     perf_mode=DR)
        two = len(routes) == 2
        for m in range(MT):
            for ri, (wt, hacc) in enumerate(routes):
                # PSUM is DVE/ACT-only territory.  Spread the combine over
                # three engines: one path is a direct DVE STT from PSUM; the
                # other bounces PSUM->SBUF through an ACT copy so the
                # (SBUF-only) gpsimd engine can do the scaled accumulate.
                w_ap = wt[m][:, n:n + 1]
                direct = (ri == 0) if two else (m == 0)
                if direct:
                    if n == 0:
                        nc.vector.tensor_scalar(hacc[m][:], pfs[m][:], w_ap,
                                                None, OP.mult)
                    else:
                        nc.vector.scalar_tensor_tensor(hacc[m][:], pfs[m][:],
                                                       w_ap, hacc[m][:],
                                                       OP.mult, OP.add)
                else:
                    if n == 0:
                        nc.scalar.activation(hacc[m][:], pfs[m][:], ACT.Copy,
                                             scale=w_ap)
                    else:
                        # gpsimd supports plain tensor_tensor but not the
                        # [p,1]-scalar ops, so ACT applies the weight
                        tmp = fp.tile([128, R], BF16, name=f"cmb{m}",
                                      tag=f"cmb{m}")
                        nc.scalar.activation(tmp[:], pfs[m][:], ACT.Copy,
                                             scale=w_ap)
                        nc.gpsimd.tensor_add(hacc[m][:], hacc[m][:], tmp[:])


def _wb_prefetch(nc, wbp, wtd, key):
    """Broadcast the full [N, TL] restore-route weight table into SBUF with
    ONE fire-and-forget DMA on the gpsimd queue, issued BEFORE any collective
    lands there — a collective blocks the queue, and dozens of per-neuron
    broadcast triggers (~0.8us queue time each) starve whatever follows."""
    wb = wbp.tile([128, N, TL], BF16, name=f"wball_{key}")
    nc.gpsimd.dma_start(wb[:], wtd[0:1, :, :].broadcast_to([128, N, TL]))
    return wb


def _restore_tok(nc, rp, gtp, rdram, routes, psy_tiles, engs, dr=True,
                 gps_ok=True):
    """Token-major restores sharing one streamed r matrix.  dr=True: fp8
    DoubleRow over adjacent kt pairs; dr=False: bf16 (the V path — fp8 r_v
    quantization is token-correlated through attention averaging and flips
    knowledge routings).  routes: list of (hT [128,4,TL] bf16, wbs: 32
    prefetched [128,TL] bf16 weight-row tiles).  psy_tiles[ri][m][db]:
    full-bank [128,512] f32 accs.  gps_ok=False when a collective was just
    issued on the gpsimd queue (ops there would stall behind it)."""
    gdt = F8 if dr else BF16
    meng = nc.gpsimd if gps_ok else nc.vector
    for pair in range(NPAIR):
        n, sub = pair // 2, pair % 2
        rc = rp.tile([128, 2, D], rdram.dtype, name="rc", tag="rc")
        engs[pair % len(engs)].dma_start(rc[:], rdram[:, 2 * pair:2 * pair + 2, :])
        for ri, (hT, wball) in enumerate(routes):
            gt = gtp.tile([128, 2, TL], gdt, name=f"gt{ri}", tag=f"gt{ri}")
            nc.vector.tensor_mul(gt[:, 0, :], hT[:, 2 * sub, :],
                                 wball[:, n, :])
            meng.tensor_mul(gt[:, 1, :], hT[:, 2 * sub + 1, :],
                            wball[:, n, :])
            if dr:
                for m in range(MT):
                    for db in range(2):
                        nc.tensor.matmul(psy_tiles[ri][m][db][:],
                                         gt[:, :, m * 128:(m + 1) * 128],
                                         rc[:, :, db * 512:(db + 1) * 512],
                                         start=(pair == 0),
                                         stop=(pair == NPAIR - 1),
                                         perf_mode=DR)
            else:
                for j in range(2):
                    for m in range(MT):
                        for db in range(2):
                            nc.tensor.matmul(
                                psy_tiles[ri][m][db][:],
                                gt[:, j, m * 128:(m + 1) * 128],
                                rc[:, j, db * 512:(db + 1) * 512],
                                start=(pair == 0 and j == 0),
                                stop=(pair == NPAIR - 1 and j == 1))


def build(dbg=False):
    nc = bacc.Bacc("TRN2", target_bir_lowering=False, debug=False,
                   num_devices=NCORES)

    x_d = nc.dram_tensor("x", [TL, D], F32, kind="ExternalInput")
    # 2 local k-blocks (straight from this core's SBUF) + 8 gathered blocks
    # (rank-major; this core's own rank fully masked to suppress duplicates)
    maskT_d = nc.dram_tensor("maskT", [(2 + SEQ_BLOCKS) * 128, TL], F32,
                             kind="ExternalInput")
    wall_d = nc.dram_tensor("wall", [128, KT, 6 * DS], F32, kind="ExternalInput")
    wo_d = nc.dram_tensor("wo", [128, KT, D], F32R, kind="ExternalInput")
    wfk_d = nc.dram_tensor("wfk", [128, KT, DS], F32, kind="ExternalInput")
    wrk_d = nc.dram_tensor("wrk", [128, KT, DS], F32, kind="ExternalInput")
    et_d = nc.dram_tensor("et", [DS, 6 * N], F32, kind="ExternalInput")
    fqk_d = nc.dram_tensor("fqk", [128, N, KT * R], F8, kind="ExternalInput")
    fv_d = nc.dram_tensor("fv", [128, N, KT * R], F8, kind="ExternalInput")
    fkn_d = nc.dram_tensor("fkn", [128, N, KT * R], F8, kind="ExternalInput")
    rqk_d = nc.dram_tensor("rqk", [128, NRT, D], F8, kind="ExternalInput")
    rv_d = nc.dram_tensor("rv", [128, NRT, D], BF16, kind="ExternalInput")
    rkn_d = nc.dram_tensor("rkn", [128, NRT, D], F8, kind="ExternalInput")
    ln_d = nc.dram_tensor("lnrows", [4, D], F32, kind="ExternalInput")
    bias_d = nc.dram_tensor("biasrow", [1, 8 * DS], F32, kind="ExternalInput")
    y_d = nc.dram_tensor("y", [TL, D], F32, kind="ExternalOutput")

    dbg_t = {}

    def dbg_tensor(name, shape):
        dbg_t[name] = nc.dram_tensor("dbg_" + name, shape, F32,
                                     kind="ExternalOutput")
        return dbg_t[name]

    with tile.TileContext(nc) as tc:
        with (
            tc.tile_pool(name="perm", bufs=1) as perm,
            tc.tile_pool(name="dramp", bufs=1, space="DRAM") as dramp,
            tc.tile_pool(name="lnp", bufs=1) as lnp,
            tc.tile_pool(name="rtp", bufs=2) as rtp,
        ):
            # one combined collective payload: K^T (KT*TL) then token-major V
            # (MT*D) — a single AllGather pays the DMA-blocking window once
            CCK = KT * TL
            cc_in = dramp.tile([128, CCK + MT * D], F8, name="cc_in")
            cc_out = dramp.tile([4 * 128, CCK + MT * D], F8, name="cc_out")
            # restore-route w rows, bounced through DRAM into one partition
            wt_dram = {k: dramp.tile([1, N, TL], BF16, name=f"wtd_{k}")
                       for k in ("rq", "rk", "rv", "rkn")}

            ident = perm.tile([128, 128], F32)
            make_identity(nc, ident[:])
            ident_b = perm.tile([128, 128], BF16)
            nc.vector.tensor_copy(ident_b[:], ident[:])
            ones_f = perm.tile([128, 1], F32)
            nc.gpsimd.memset(ones_f[:], 1.0)
            ones_b = perm.tile([128, 1], BF16)
            nc.vector.tensor_copy(ones_b[:], ones_f[:])
            psc = perm.tile([128, 1], F32)
            nc.gpsimd.memset(psc[:], PSUM_SCALE)
            bias_bc = perm.tile([128, 8 * DS], F32)
            nc.sync.dma_start(bias_bc[:], bias_d[0:1, :].broadcast_to([128, 8 * DS]))
            et_sb = perm.tile([DS, 6 * N], F32)
            nc.sync.dma_start(et_sb[:], et_d[:])
            # copy at partition base 64 for routings whose h-segment sits in
            # the upper half of a transposed tile (matmul requires equal bases)
            et_hi = perm.tile([128, 6 * N], F32)
            nc.sync.dma_start(et_hi[DS:2 * DS, :], et_d[:])
            x_sb = perm.tile([128, MT, D], F32)
            for m in range(MT):
                nc.sync.dma_start(x_sb[:, m, :], x_d[m * 128:(m + 1) * 128, :])
            maskT_sb = perm.tile([128, 2 + SEQ_BLOCKS, TL], F32)
            yT_q = perm.tile([128, KT, TL], F8)
            yT_k = perm.tile([128, KT, TL], F8)
            v_tok = perm.tile([128, MT, D], F8)

            # ============ stage 1: LN1 + routing + features + restores ========
            with (
                tc.tile_pool(name="st1", bufs=1) as st1,
                tc.tile_pool(name="fchunk", bufs=4) as fp,
                tc.tile_pool(name="rchunk", bufs=4) as rp,
                tc.tile_pool(name="gtp", bufs=3) as gtp,
            ):
                nxT_r = st1.tile([128, KT, TL], F8)
                h_q = [st1.tile([128, R], BF16, name=f"h_q{m}") for m in range(MT)]
                h_k = [st1.tile([128, R], BF16, name=f"h_k{m}") for m in range(MT)]
                h_v = [st1.tile([128, R], BF16, name=f"h_v{m}") for m in range(MT)]
                hT = {k: st1.tile([128, 4, TL], BF16, name=f"hT_{k}")
                      for k in ("q", "k", "v")}
                w_feat = {p: [st1.tile([128, N], F32, name=f"w{p}_{m}")
                              for m in range(MT)] for p in range(3)}
                wtt_sb = {k: st1.tile([N, TL], BF16, name=f"wtt_{k}")
                          for k in ("rq", "rk", "rv")}
                kst = st1.tile([128, MT, D], BF16, name="kst")
                qst = st1.tile([128, MT, D], BF16, name="qst")

                with (
                    tc.tile_pool(name="st1a", bufs=1) as st1a,
                    tc.tile_pool(name="ps_tr", bufs=2, space="PSUM") as pstr,
                    tc.tile_pool(name="ps_mm", bufs=2, space="PSUM") as psmm,
                    tc.tile_pool(name="ps_feat", bufs=2, space="PSUM") as psf,
                    tc.tile_pool(name="wallp", bufs=2) as wallp,
                ):
                    nxT = st1a.tile([128, KT, TL], F32)
                    nx = st1a.tile([128, MT, D], F32)
                    ln1_bc = st1a.tile([128, 2, D], F32)
                    for i in range(2):
                        nc.gpsimd.dma_start(ln1_bc[:, i, :],
                                            ln_d[i:i + 1, :]
                                            .broadcast_to([128, D]))
                    for m in range(MT):
                        _layernorm(nc, lnp, f"ln1_{m}", x_sb[:, m, :],
                                   ln1_bc[:, 0, :], ln1_bc[:, 1, :], nx[:, m, :])
                    for m in range(MT):
                        for k in range(KT):
                            _tr(nc, pstr, f"nx_{m}_{k}",
                                nx[:, m, k * 128:(k + 1) * 128], ident[:],
                                [(nxT[:, k, m * 128:(m + 1) * 128], "v", None),
                                 (nxT_r[:, k, m * 128:(m + 1) * 128], "s",
                                  S_NX)])

                    hall = st1a.tile([128, MT, 6 * DS], F32)
                    phs = [psmm.tile([128, 6 * DS], F32, name=f"ph{m}",
                                     tag="mm") for m in range(MT)]
                    for k in range(KT):
                        wt_k = wallp.tile([128, 6 * DS], F32, name="wal",
                                          tag="wal")
                        [nc.sync, nc.scalar][k % 2].dma_start(
                            wt_k[:], wall_d[:, k, :])
                        for m in range(MT):
                            nc.tensor.matmul(phs[m][:],
                                             nxT[:, k, m * 128:(m + 1) * 128],
                                             wt_k[:],
                                             start=(k == 0), stop=(k == KT - 1))
                    for m in range(MT):
                        nc.vector.tensor_add(hall[:, m, :], phs[m][:],
                                             bias_bc[:, :6 * DS])
                    # prefetch the attention mask well before stage 2
                    for kb in range(2 + SEQ_BLOCKS):
                        nc.gpsimd.dma_start(maskT_sb[:, kb, :],
                                            maskT_d[kb * 128:(kb + 1) * 128, :])
                    hallT = st1a.tile([128, 3, TL], F32)
                    for m in range(MT):
                        for i in range(3):
                            _tr(nc, pstr, f"ha_{m}_{i}",
                                hall[:, m, i * 128:(i + 1) * 128], ident[:],
                                [(hallT[:, i, m * 128:(m + 1) * 128], "v",
                                  None)])
                    w_rest = {}
                    for p in range(6):
                        seg = ATTN_SEG[p]
                        tiles = w_feat[p] if p < 3 else \
                            [st1a.tile([128, N], F32, name=f"w{p}_{m}")
                             for m in range(MT)]
                        if p >= 3:
                            w_rest[p] = tiles
                        for m in range(MT):
                            base, ti = (p % 2) * DS, p // 2
                            e_src = et_sb if base == 0 else et_hi
                            e_ap = e_src[base:base + DS,
                                         seg * N:(seg + 1) * N]
                            _routing(nc, rtp, psmm, f"r{p}_{m}",
                                     hallT[base:base + DS, ti,
                                           m * 128:(m + 1) * 128],
                                     e_ap, tiles[m][:])
                    wbsets = {}
                    for p, key in [(3, "rq"), (4, "rk"), (5, "rv")]:
                        for m in range(MT):
                            _tr(nc, pstr, f"wt_{p}_{m}", w_rest[p][m][:],
                                ident[:],
                                [(wtt_sb[key][:, m * 128:(m + 1) * 128], "v",
                                  WB_SCALE)])
                        nc.gpsimd.dma_start(wt_dram[key][0], wtt_sb[key][:])
                    # all 96 weight-row broadcasts issued here, before any
                    # collective lands on the gpsimd queue
                    for key in ("rk", "rv", "rq"):
                        wbsets[key] = _wb_prefetch(nc, st1, wt_dram[key], key)

                    # features (qk shared for Q and K; v)
                    _feature(nc, fp, psf, fqk_d, nxT_r,
                             [(w_feat[0], h_q), (w_feat[1], h_k)],
                             [nc.sync, nc.scalar])
                    _feature(nc, fp, psf, fv_d, nxT_r,
                             [(w_feat[2], h_v)], [nc.sync, nc.scalar])
                    for nm, hh in [("q", h_q), ("k", h_k), ("v", h_v)]:
                        for m in range(MT):
                            for rb in range(4):
                                _tr(nc, pstr, f"h{nm}_{m}_{rb}",
                                    hh[m][:, rb * 128:(rb + 1) * 128], ident_b[:],
                                    [(hT[nm][:, rb, m * 128:(m + 1) * 128],
                                      "v", None)])

                # restores: Q+K fused (one r_qk stream), then V; the single
                # combined AllGather goes out after ALL weight streaming —
                # any DMA issued after a collective waits for it to complete,
                # so nothing DMA-hungry may follow.  Q's transposes (DMA-free)
                # and the local-K part of phase A absorb the gather.
                def _stage_out(ps_tiles, st, scale):
                    for m in range(MT):
                        for db in range(2):
                            dst = st[:, m, db * 512:(db + 1) * 512]
                            if (m + db) % 2 == 0:
                                nc.scalar.activation(dst, ps_tiles[m][db][:],
                                                     ACT.Copy, scale=scale)
                            else:
                                nc.vector.tensor_scalar_mul(
                                    dst, ps_tiles[m][db][:], scale)

                with tc.tile_pool(name="ps_y", bufs=1, space="PSUM") as psy:
                    pqk = [[[psy.tile([128, 512], F32, name=f"py{ri}{m}{db}")
                             for db in range(2)] for m in range(MT)]
                           for ri in range(2)]
                    _restore_tok(nc, rp, gtp, rqk_d,
                                 [(hT["q"][:], wbsets["rq"]),
                                  (hT["k"][:], wbsets["rk"])],
                                 pqk, [nc.sync, nc.scalar])
                    _stage_out(pqk[1], kst, STAGE_SCALE)
                    _stage_out(pqk[0], qst, STAGE_SCALE)
                with tc.tile_pool(name="ps_tr2", bufs=2, space="PSUM") as pstr2:
                    for m in range(MT):
                        for dt in range(KT):
                            _tr(nc, pstr2, f"kT_{m}_{dt}",
                                kst[:, m, dt * 128:(dt + 1) * 128], ident_b[:],
                                [(yT_k[:, dt, m * 128:(m + 1) * 128], "v",
                                  None)])
                    nc.sync.dma_start(cc_in[:, :CCK],
                                      yT_k[:].rearrange("p k t -> p (k t)"))
                # V restore, token-major straight into its collective payload
                with tc.tile_pool(name="ps_yv", bufs=1, space="PSUM") as psy:
                    pvs = [[psy.tile([128, 512], F32, name=f"pv{m}{db}")
                            for db in range(2)] for m in range(MT)]
                    _restore_tok(nc, rp, gtp, rv_d,
                                 [(hT["v"][:], wbsets["rv"])],
                                 [pvs], [nc.sync, nc.scalar], dr=False)
                    _stage_out(pvs, v_tok, STAGE_SCALE)
                for m in range(MT):
                    nc.sync.dma_start(cc_in[:, CCK + m * D:CCK + (m + 1) * D],
                                      v_tok[:, m, :])
                nc.gpsimd.collective_compute(
                    "AllGather", OP.bypass,
                    ins=[cc_in[:]],
                    outs=[cc_out[:]],
                    replica_groups=[[0, 1, 2, 3], [4, 5, 6, 7]],
                )
                # DMA-free while the gather flies
                with tc.tile_pool(name="ps_tr2b", bufs=2, space="PSUM") as pstr2:
                    for m in range(MT):
                        for dt in range(KT):
                            _tr(nc, pstr2, f"qT_{m}_{dt}",
                                qst[:, m, dt * 128:(dt + 1) * 128], ident_b[:],
                                [(yT_q[:, dt, m * 128:(m + 1) * 128], "s",
                                  None)])

            # ============ stage 2: attention + W_o ============
            late_cm = tc.tile_pool(name="late", bufs=1)
            late = late_cm.__enter__()
            x2 = late.tile([128, MT, D], F32)
            ot_sb = late.tile([128, KT, TL], F32)
            with (
                tc.tile_pool(name="st2", bufs=1) as st2,
                tc.tile_pool(name="attp", bufs=3) as att,
                tc.tile_pool(name="ps_att", bufs=2, space="PSUM") as psa,
                tc.tile_pool(name="ps_ot", bufs=4, space="PSUM") as psot,
            ):
                # phase A1: scores for this core's own 2 k-blocks straight
                # from SBUF (yT_k) — DMA-free work that hides the AllGather.
                NB = 2 + SEQ_BLOCKS
                expt_all = st2.tile([128, 2 * KT, NB, TL], F8)
                rbc_all = st2.tile([DH, H, TL], F32)
                sums_sb = st2.tile([1, H, TL], F32)
                for hp in range(KT):
                    for hh in range(2):
                        h_idx = hp * 2 + hh
                        qt_ap = yT_q[hh * DH:(hh + 1) * DH, hp, :]
                        msc2 = att.tile([128, 2, TL], F32, name="msc2",
                                        tag="msc2")
                        for b in range(2):
                            ktap = yT_k[hh * DH:(hh + 1) * DH, hp,
                                        b * 128:(b + 1) * 128]
                            pscore = psa.tile([128, TL], F32, name="pscore",
                                              tag="pscore")
                            nc.tensor.matmul(pscore[:], ktap, qt_ap,
                                             start=True, stop=True)
                            nc.vector.tensor_add(msc2[:, b, :], pscore[:],
                                                 maskT_sb[:, b, :])
                        nc.scalar.activation(expt_all[:, h_idx, 0:2, :],
                                             msc2[:], ACT.Exp,
                                             scale=EXP_SCALE)
                # phase A2: gathered blocks + softmax denominators.  Exps run
                # 4 k-blocks per ACT instruction; denominator matmuls lag one
                # quarter so the PE never waits on the exp chain.
                with tc.tile_pool(name="ktp", bufs=1) as ktp:
                    kt_all = ktp.tile([128, 4, KT * TL], F8)
                    ld_engs = [nc.sync, nc.scalar]
                    for ch in range(4):
                        ld_engs[ch % 2].dma_start(
                            kt_all[:, ch, :],
                            cc_out[ch * 128:(ch + 1) * 128, :CCK])
                    for hp in range(KT):
                        for hh in range(2):
                            h_idx = hp * 2 + hh
                            qt_ap = yT_q[hh * DH:(hh + 1) * DH, hp, :]
                            pss = psa.tile([1, TL], F32, name="pss",
                                           tag="pss")
                            nc.vector.memset(pss[:], 0.0)
                            for kbq in range(2):
                                msc4 = att.tile([128, 4, TL], F32,
                                                name="msc4", tag="msc")
                                for j in range(4):
                                    gb = kbq * 4 + j
                                    b = 2 + gb
                                    ch, m2 = gb // 2, gb % 2
                                    ktap = kt_all[hh * DH:(hh + 1) * DH, ch,
                                                  hp * TL + m2 * 128:
                                                  hp * TL + (m2 + 1) * 128]
                                    pscore = psa.tile([128, TL], F32,
                                                      name="pscore",
                                                      tag="pscore")
                                    nc.tensor.matmul(pscore[:], ktap, qt_ap,
                                                     start=True, stop=True)
                                    # spread the mask adds: DVE direct from
                                    # PSUM, or ACT copy-out + gpsimd add
                                    if gb % 2 == 0:
                                        nc.vector.tensor_add(
                                            msc4[:, j, :], pscore[:],
                                            maskT_sb[:, b, :])
                                    else:
                                        mtmp = att.tile([128, TL], F32,
                                                        name="mtmp",
                                                        tag="mtmp")
                                        nc.scalar.copy(mtmp[:], pscore[:])
                                        nc.gpsimd.tensor_add(
                                            msc4[:, j, :], mtmp[:],
                                            maskT_sb[:, b, :])
                                nc.scalar.activation(
                                    expt_all[:, h_idx,
                                             2 + 4 * kbq:2 + 4 * kbq + 4, :],
                                    msc4[:], ACT.Exp, scale=EXP_SCALE)
                                if kbq == 1:
                                    for b in range(6):
                                        nc.tensor.matmul(
                                            pss[:], ones_b[:],
                                            expt_all[:, h_idx, b, :],
                                            start=False, stop=False)
                            for b in range(6, NB):
                                nc.tensor.matmul(
                                    pss[:], ones_b[:],
                                    expt_all[:, h_idx, b, :],
                                    start=False, stop=(b == NB - 1))
                            nc.vector.tensor_copy(sums_sb[0:1, h_idx, :],
                                                  pss[:])
                    # DVE reciprocal costs ~15 cyc/element on ONE partition's
                    # free dim, so respread all 16 heads' sums across 128
                    # partitions via DMA, reciprocate there (~30 elem each),
                    # and DMA back before the per-head broadcasts.
                    spr = st2.tile([128, H * TL // 128], F32)
                    spr_r = st2.tile([128, H * TL // 128], F32)
                    rinv = st2.tile([1, H, TL], F32)
                    nc.sync.dma_start(
                        spr[:], sums_sb[:].rearrange("p h t -> p (h t)"))
                    nc.vector.reciprocal(spr_r[:], spr[:])
                    nc.sync.dma_start(
                        rinv[:].rearrange("p h t -> p (h t)"), spr_r[:])
                    for h_idx in range(H):
                        nc.gpsimd.partition_broadcast(
                            rbc_all[:, h_idx, :], rinv[0:1, h_idx, :],
                            channels=DH)
                # phase B: AV — a dense run of matmuls (local V from SBUF)
                v_all = st2.tile([128, 4, MT * D], F8)
                ld_engs = [nc.sync, nc.scalar, nc.gpsimd]
                for ch in range(4):
                    ld_engs[ch % 3].dma_start(
                        v_all[:, ch, :],
                        cc_out[ch * 128:(ch + 1) * 128, CCK:])
                for hp in range(KT):
                    for hh in range(2):
                        h_idx = hp * 2 + hh
                        pot = psot.tile([DH, TL], F32, name="pot", tag="pot")
                        nc.vector.memset(pot[:], 0.0)
                        for b in range(NB):
                            if b < 2:
                                vap = v_tok[:, b,
                                            h_idx * DH:(h_idx + 1) * DH]
                            else:
                                gb = b - 2
                                ch, m2 = gb // 2, gb % 2
                                vap = v_all[:, ch,
                                            m2 * D + h_idx * DH:
                                            m2 * D + (h_idx + 1) * DH]
                            nc.tensor.matmul(pot[:], vap,
                                             expt_all[:, h_idx, b, :],
                                             start=False,
                                             stop=(b == NB - 1))
                        otn = att.tile([DH, TL], F32, name="otn", tag="otn")
                        nc.vector.tensor_mul(otn[:], pot[:],
                                             rbc_all[:, h_idx, :])
                        # SBUF->SBUF DMA can shift partitions (DVE cannot)
                        nc.sync.dma_start(ot_sb[hh * DH:(hh + 1) * DH, hp, :],
                                          otn[:])

            with (
                tc.tile_pool(name="wop", bufs=3) as wop,
                tc.tile_pool(name="ps_mm2", bufs=2, space="PSUM") as psmm2,
            ):
                ot_r = ot_sb[:].bitcast(F32R)
                for blk in range(2):
                    wo_t = []
                    for k in range(KT):
                        wt_k = wop.tile([128, 512], F32R, name=f"wo{k}",
                                        tag="wo")
                        [nc.sync, nc.scalar][k % 2].dma_start(
                            wt_k[:], wo_d[:, k, blk * 512:(blk + 1) * 512])
                        wo_t.append(wt_k)
                    for m in range(MT):
                        px = psmm2.tile([128, 512], F32, name="px", tag="mm")
                        for k in range(KT):
                            nc.tensor.matmul(px[:],
                                             ot_r[:, k, m * 128:(m + 1) * 128],
                                             wo_t[k][:],
                                             start=(k == 0), stop=(k == KT - 1))
                        nc.vector.tensor_add(
                            x2[:, m, blk * 512:(blk + 1) * 512], px[:],
                            x_sb[:, m, blk * 512:(blk + 1) * 512])
            if dbg:
                td = dbg_tensor("x2", [TL, D])
                for m in range(MT):
                    nc.sync.dma_start(td[m * 128:(m + 1) * 128, :], x2[:, m, :])

            # ============ stage 3: knowledge circuit ============
            with (
                tc.tile_pool(name="st3", bufs=1) as st3,
                tc.tile_pool(name="fchunk2", bufs=3) as fp2,
                tc.tile_pool(name="rchunk2", bufs=3) as rp2,
                tc.tile_pool(name="gtp2", bufs=3) as gtp2,
            ):
                nx2T_r = st3.tile([128, KT, TL], F8)
                h_kn = [st3.tile([128, R], BF16, name=f"h_kn{m}")
                        for m in range(MT)]
                hT_kn = st3.tile([128, 4, TL], BF16)
                wtt_kn = st3.tile([N, TL], BF16, name="wtt_kn")
                w_kn = {}
                with (
                    tc.tile_pool(name="st3a", bufs=1) as st3a,
                    tc.tile_pool(name="ps_tr3", bufs=2, space="PSUM") as pstr3,
                    tc.tile_pool(name="ps_mm3", bufs=2, space="PSUM") as psmm3,
                    tc.tile_pool(name="ps_feat3", bufs=2, space="PSUM") as psf3,
                ):
                    nx2 = st3a.tile([128, MT, D], F32)
                    ln2_bc = st3a.tile([128, 2, D], F32)
                    for i in range(2):
                        nc.sync.dma_start(
                            ln2_bc[:, i, :],
                            ln_d[i + 2:i + 3, :].broadcast_to([128, D]))
                    for m in range(MT):
                        _layernorm(nc, lnp, f"ln2_{m}", x2[:, m, :],
                                   ln2_bc[:, 0, :], ln2_bc[:, 1, :], nx2[:, m, :])
                    nx2T = st3a.tile([128, KT, TL], F32)
                    for m in range(MT):
                        for k in range(KT):
                            _tr(nc, pstr3, f"nx2_{m}_{k}",
                                nx2[:, m, k * 128:(k + 1) * 128], ident[:],
                                [(nx2T[:, k, m * 128:(m + 1) * 128], "v", None),
                                 (nx2T_r[:, k, m * 128:(m + 1) * 128], "s",
                                  S_NX)])
                    wk_sb = st3a.tile([128, KT, 2 * DS], F32)
                    nc.sync.dma_start(wk_sb[:, :, :DS], wfk_d[:])
                    nc.sync.dma_start(wk_sb[:, :, DS:], wrk_d[:])
                    hkT = st3a.tile([DS, 2, TL], F32)
                    for m in range(MT):
                        for j in range(2):
                            pk = psmm3.tile([128, DS], F32, name="pk", tag="mm")
                            for k in range(KT):
                                nc.tensor.matmul(
                                    pk[:], nx2T[:, k, m * 128:(m + 1) * 128],
                                    wk_sb[:, k, j * DS:(j + 1) * DS],
                                    start=(k == 0), stop=(k == KT - 1))
                            hk = rtp.tile([128, DS], F32, name=f"hk{m}{j}",
                                          tag="hk")
                            nc.vector.tensor_add(
                                hk[:], pk[:],
                                bias_bc[:, (6 + j) * DS:(7 + j) * DS])
                            _tr(nc, pstr3, f"hk_{m}_{j}", hk[:], ident[:],
                                [(hkT[:, j, m * 128:(m + 1) * 128], "v",
                                  None)])
                    for j, nm in [(0, "fkn"), (1, "rkn")]:
                        w_kn[nm] = []
                        for m in range(MT):
                            wt = st3.tile([128, N], F32, name=f"wkn{j}_{m}")
                            _routing(nc, rtp, psmm3, f"rk{j}_{m}",
                                     hkT[:, j, m * 128:(m + 1) * 128],
                                     et_sb[:, (4 + j) * N:(5 + j) * N], wt[:])
                            w_kn[nm].append(wt)
                    for m in range(MT):
                        _tr(nc, pstr3, f"wt_kn_{m}", w_kn["rkn"][m][:],
                            ident[:],
                            [(wtt_kn[:, m * 128:(m + 1) * 128], "v",
                              WB_SCALE)])
                    nc.gpsimd.dma_start(wt_dram["rkn"][0], wtt_kn[:])
                    wbs_kn = _wb_prefetch(nc, st3, wt_dram["rkn"], "rkn")

                    _feature(nc, fp2, psf3, fkn_d, nx2T_r,
                             [(w_kn["fkn"], h_kn)],
                             [nc.sync, nc.scalar])
                    for m in range(MT):
                        for rb in range(4):
                            _tr(nc, pstr3, f"hkn_{m}_{rb}",
                                h_kn[m][:, rb * 128:(rb + 1) * 128], ident_b[:],
                                [(hT_kn[:, rb, m * 128:(m + 1) * 128], "v",
                                  None)])

                out_sb = st3.tile([128, MT, D], F32)
                with tc.tile_pool(name="ps_y3", bufs=1, space="PSUM") as psy3:
                    pkn = [[[psy3.tile([128, 512], F32, name=f"pn{m}{db}")
                             for db in range(2)] for m in range(MT)]]
                    _restore_tok(nc, rp2, gtp2, rkn_d,
                                 [(hT_kn[:], wbs_kn)],
                                 pkn, [nc.sync, nc.scalar])
                    for m in range(MT):
                        for db in range(2):
                            nc.vector.scalar_tensor_tensor(
                                out_sb[:, m, db * 512:(db + 1) * 512],
                                pkn[0][m][db][:], psc[:],
                                x2[:, m, db * 512:(db + 1) * 512],
                                OP.mult, OP.add)
                for m in range(MT):
                    nc.sync.dma_start(y_d[m * 128:(m + 1) * 128, :],
                                      out_sb[:, m, :])
            late_cm.__exit__(None, None, None)

    nc.compile()
    return nc, dbg_t


def prep_inputs(inputs):
    f32 = np.float32
    fp8 = mybir.dt.np(F8)
    x = np.ascontiguousarray(np.asarray(inputs["x"], f32).reshape(T, D))
    ne = np.asarray(inputs["neuron_emb"], f32)
    emb = ne / (np.linalg.norm(ne, axis=-1, keepdims=True) + 1e-8)

    def f_layout(f):
        f = np.asarray(f, f32) * S_W
        return np.ascontiguousarray(
            f.reshape(N, KT, 128, R).transpose(2, 0, 1, 3)
            .reshape(128, N, KT * R).astype(fp8))

    def r_layout(r, dt=None):
        r = np.asarray(r, f32).reshape(N * R, D) * S_W
        return np.ascontiguousarray(
            r.reshape(NRT, 128, D).transpose(1, 0, 2).astype(dt or fp8))

    def w_layout(w, pre=1.0):
        w = np.asarray(w, f32) * pre
        return np.ascontiguousarray(
            w.reshape(KT, 128, w.shape[-1]).transpose(1, 0, 2))

    shared = {
        "wall": w_layout(inputs["W_all"]),
        "wo": w_layout(inputs["W_o"], WO_PRE),
        "wfk": w_layout(inputs["W_fk"]),
        "wrk": w_layout(inputs["W_rk"]),
        "et": np.ascontiguousarray(emb.T),
        "fqk": f_layout(inputs["f_qk"]),
        "fv": f_layout(inputs["f_v"]),
        "fkn": f_layout(inputs["f_know"]),
        "rqk": r_layout(inputs["r_qk"]),
        "rv": r_layout(inputs["r_v"], mybir.dt.np(BF16)),
        "rkn": r_layout(inputs["r_know"]),
        "lnrows": np.ascontiguousarray(
            np.stack([np.asarray(inputs[k], f32)
                      for k in ("ln1_s", "ln1_b", "ln2_s", "ln2_b")])),
        "biasrow": np.ascontiguousarray(
            np.concatenate([np.asarray(inputs["b_all"], f32),
                            np.asarray(inputs["b_fk"], f32),
                            np.asarray(inputs["b_rk"], f32)])[None, :]),
    }
    per_core = []
    for c in range(NCORES):
        ci = c % (S // TL)
        q_idx = ci * TL + np.arange(TL)[None, :]
        # block layout: 2 local k-blocks (this core's tokens, from SBUF) then
        # 8 gathered blocks in rank-major order; this core's own rank is
        # fully masked in the gathered set to suppress the duplicates.
        blocks = []
        for b in range(2):
            k_idx = ci * TL + b * 128 + np.arange(128)[:, None]
            blocks.append(np.where(k_idx <= q_idx, 0.0, NEG))
        for r in range(4):
            for m2 in range(2):
                k_idx = r * TL + m2 * 128 + np.arange(128)[:, None]
                blk = np.where(k_idx <= q_idx, 0.0, NEG)
                if r == ci:
                    blk = np.full((128, TL), NEG)
                blocks.append(blk)
        maskT = np.concatenate(blocks, axis=0).astype(f32)
        per_core.append({
            "x": np.ascontiguousarray(x[c * TL:(c + 1) * TL]),
            "maskT": np.ascontiguousarray(maskT),
            **shared,
        })
    return per_core


def kernel(**inputs):
    global _PROG
    if _PROG is None:
        _PROG = build(dbg=False)
    nc, _ = _PROG
    per_core = prep_inputs(inputs)
    res = run_bass_kernel_spmd(nc, per_core, core_ids=list(range(NCORES)))
    y = np.concatenate([res.results[c]["y"] for c in range(NCORES)], axis=0)
    return y.reshape(B, S, D).astype(np.float32)
